# revision 1
# baseline (speedup 1.0000x reference)
"""Gaussian-kernel matrix on 8 Trainium2 NeuronCores.

Math (identical factorization to the reference):
    dist(f)[n,k] = -sum_c ((f[n,c]-means[k,c])/scales[k,c])^2
                 = -(f^2 @ g.T) + 2*(f @ (means*g).T) - const[k],
      where g = 1/scales^2, const[k] = sum_c means[k,c]^2 g[k,c]
    out = (exp(dist_i) * weights) @ exp(dist_j).T

Sharding: 2D grid (4 f_i-blocks x 2 f_j-blocks) over 8 cores; each core
computes an independent [2048, 4096] output block.

Device kernel (per core), all matmuls bf16 / fp32-accumulate:
  - dist matmuls run 2x column-tiled (PE tiles (0,0)/(0,64)): dist_j packs
    two output n-chunks per PSUM bank; dist_i duplicates its result into
    both partition halves (needed by the row-tiled main matmul).
  - main matmul runs 2x row-tiled (PE tiles (0,0)/(64,0)): the K=64
    contraction only fills half the array, so two output tiles run
    concurrently, one per array half.
  - exp on ScalarE with the -const[k] bias applied per-partition.
"""

import numpy as np
import ml_dtypes

import concourse.bacc as bacc
import concourse.mybir as mybir
import concourse.tile as tile
from concourse.bass_utils import run_bass_kernel_spmd

N, C, K = 8192, 512, 64
R, Q = 4, 2                 # f_i split x f_j split
MI, MJ = N // R, N // Q     # 2048, 4096 rows per core
NCH = 512                   # matmul free-dim / psum bank (fp32)
CT = C // 128               # 4 partition tiles of the feature dim
SI, SJ = MI // NCH, MJ // (2 * NCH)   # dist_i chunks (4), dist_j slot pairs (4)

F32 = mybir.dt.float32
BF16 = mybir.dt.bfloat16
FP8 = mybir.dt.float8e4
BF16_NP = ml_dtypes.bfloat16
FP8_NP = ml_dtypes.float8_e4m3
Exp = mybir.ActivationFunctionType.Exp
Square = mybir.ActivationFunctionType.Square


def build_nc(iters: int = 1, merge_small: bool = True, split_rows: int = 2):
    """Build + compile the per-core Bass graph.  iters>1 wraps the body in a
    runtime loop (used only for wall-clock benchmarking)."""
    nc = bacc.Bacc("TRN2", target_bir_lowering=False)

    fiT_ext = nc.declare_dram_parameter("fiT", [C, MI], FP8, isOutput=False)
    fjT_ext = nc.declare_dram_parameter("fjT", [C, MJ], FP8, isOutput=False)
    # means/scales host-retiled to [128, CT*K] (c-chunks along free dim),
    # packed with the dup'd weight column into one small tensor
    SMALL = 2 * CT * K + 1
    small_ext = nc.declare_dram_parameter("small", [128, SMALL], F32, isOutput=False)
    out_ext = nc.declare_dram_parameter("out", [MI, MJ], F32, isOutput=True)

    with tile.TileContext(nc) as tc:
        with (
            tc.tile_pool(name="persist", bufs=1) as persist,
            tc.tile_pool(name="scratch", bufs=2) as scratch,
            tc.tile_pool(name="stage", bufs=3) as stage,
            tc.tile_pool(name="psum", bufs=1, space="PSUM") as psum,
        ):

            def body():
                # ---- input DMAs: small packed tensor first, then fiT, then fjT ----
                small = persist.tile([128, SMALL], F32, name="small", tag="small")
                nc.sync.dma_start(small[:], small_ext[:])
                meansT2 = small[:, 0:CT * K]
                scalesT2 = small[:, CT * K:2 * CT * K]
                w2 = small[:, 2 * CT * K:SMALL]
                fiT = [persist.tile([128, MI], FP8, name=f"fiT{c}", tag=f"fiT{c}")
                       for c in range(CT)]
                fjT = [persist.tile([128, MJ], FP8, name=f"fjT{c}", tag=f"fjT{c}")
                       for c in range(CT)]
                for c in range(CT):
                    nc.sync.dma_start(fiT[c][:], fiT_ext[c * 128:(c + 1) * 128, :])
                for c in range(CT):
                    nc.sync.dma_start(fjT[c][:], fjT_ext[c * 128:(c + 1) * 128, :])

                # ---- per-chunk weights: -g, 2*means*g, means^2*g (bf16) ----
                negg, mg2, m2g = [], [], []
                for c in range(CT):
                    msl_ = slice(c * K, (c + 1) * K)
                    ssl_ = slice(CT * K + c * K, CT * K + (c + 1) * K)
                    sq = scratch.tile([128, K], F32, name="sq", tag="sq")
                    nc.vector.tensor_mul(sq[:], small[:, ssl_], small[:, ssl_])
                    rec = scratch.tile([128, K], F32, name="rec", tag="rec")
                    nc.vector.reciprocal(rec[:], sq[:])
                    ng = persist.tile([128, K], FP8, name=f"negg{c}", tag=f"negg{c}")
                    nc.vector.tensor_scalar_mul(ng[:], rec[:], -1.0)
                    mg = scratch.tile([128, K], F32, name="mg", tag="mg")
                    nc.vector.tensor_mul(mg[:], small[:, msl_], rec[:])
                    m2 = persist.tile([128, K], FP8, name=f"mg2_{c}", tag=f"mg2_{c}")
                    nc.vector.tensor_scalar_mul(m2[:], mg[:], 2.0)
                    mm = persist.tile([128, K], BF16, name=f"m2g{c}", tag=f"m2g{c}")
                    nc.vector.tensor_mul(mm[:], small[:, msl_], mg[:])
                    negg.append(ng)
                    mg2.append(m2)
                    m2g.append(mm)

                # ---- const[k] into both psum halves (col-tiled), bias = -const ----
                ones = persist.tile([128, 1], BF16, name="ones", tag="ones")
                nc.vector.memset(ones[:], 1.0)
                cps = psum.tile([128, 1], F32, name="cps", tag="dpsi", bufs=2)
                for c in range(CT):
                    nc.tensor.matmul(cps[0:64, :], m2g[c][:], ones[:],
                                     start=(c == 0), stop=(c == CT - 1),
                                     tile_position=(0, 0))
                    nc.tensor.matmul(cps[64:128, :], m2g[c][:], ones[:],
                                     start=(c == 0), stop=(c == CT - 1),
                                     tile_position=(0, 64))
                bias = persist.tile([128, 1], F32, name="bias", tag="bias")
                nc.vector.tensor_scalar_mul(bias[:], cps[:], -1.0)

                # ---- squared features (bf16): all f_i first (its DMAs land first) ----
                f2iT = [persist.tile([128, MI], FP8, name=f"f2iT{c}", tag=f"f2iT{c}")
                        for c in range(CT)]
                f2jT = [persist.tile([128, MJ], FP8, name=f"f2jT{c}", tag=f"f2jT{c}")
                        for c in range(CT)]
                for c in range(CT):
                    h = MI // 2
                    nc.vector.tensor_mul(f2iT[c][:, 0:h], fiT[c][:, 0:h], fiT[c][:, 0:h])
                    nc.scalar.activation(f2iT[c][:, h:MI], fiT[c][:, h:MI], Square)
                for c in range(CT):
                    q = MJ // 4
                    for s in range(4):
                        qsl = slice(s * q, (s + 1) * q)
                        if s % 2 == 0:
                            nc.vector.tensor_mul(f2jT[c][:, qsl], fjT[c][:, qsl],
                                                 fjT[c][:, qsl])
                        else:
                            nc.scalar.activation(f2jT[c][:, qsl], fjT[c][:, qsl], Square)

                # ---- phi_i (both halves identical): [128, MI] bf16 ----
                # dist_i n-outer, col-tiled duplicate into both psum halves
                phi_i2 = persist.tile([128, MI], BF16, name="phi_i2", tag="phi_i2")
                for n in range(SI):
                    sl = slice(n * NCH, (n + 1) * NCH)
                    ps = psum.tile([128, NCH], F32, name="dpsi", tag="dpsi", bufs=2)
                    for c in range(CT):
                        nc.tensor.matmul(ps[0:64, :], negg[c][:], f2iT[c][:, sl],
                                         start=(c == 0), stop=False,
                                         tile_position=(0, 0))
                        nc.tensor.matmul(ps[64:128, :], negg[c][:], f2iT[c][:, sl],
                                         start=(c == 0), stop=False,
                                         tile_position=(0, 64))
                    for c in range(CT):
                        nc.tensor.matmul(ps[0:64, :], mg2[c][:], fiT[c][:, sl],
                                         start=False, stop=(c == CT - 1),
                                         tile_position=(0, 0))
                        nc.tensor.matmul(ps[64:128, :], mg2[c][:], fiT[c][:, sl],
                                         start=False, stop=(c == CT - 1),
                                         tile_position=(0, 64))
                    ex = scratch.tile([128, NCH], F32, name="ex", tag="ex")
                    nc.scalar.activation(ex[:], ps[:], Exp, bias=bias[:], scale=1.0)
                    nc.vector.tensor_scalar_mul(phi_i2[:, sl], ex[:], w2)

                # ---- phi_j packed: [128, MJ/2] bf16; half0 = even chunks, half1 = odd ----
                # c-outer accumulation so PE streams as fjT chunks land
                phi_j2 = persist.tile([128, MJ // 2], BF16, name="phi_j2", tag="phi_j2")
                psj = [psum.tile([128, NCH], F32, name=f"dpsj{s}", tag=f"dpsj{s}")
                       for s in range(SJ)]
                for c in range(CT):
                    for s in range(SJ):
                        ev = slice((2 * s) * NCH, (2 * s + 1) * NCH)
                        od = slice((2 * s + 1) * NCH, (2 * s + 2) * NCH)
                        nc.tensor.matmul(psj[s][0:64, :], negg[c][:], f2jT[c][:, ev],
                                         start=(c == 0), stop=False,
                                         tile_position=(0, 0))
                        nc.tensor.matmul(psj[s][64:128, :], negg[c][:], f2jT[c][:, od],
                                         start=(c == 0), stop=False,
                                         tile_position=(0, 64))
                        nc.tensor.matmul(psj[s][0:64, :], mg2[c][:], fjT[c][:, ev],
                                         start=False, stop=(c == CT - 1),
                                         tile_position=(0, 0))
                        nc.tensor.matmul(psj[s][64:128, :], mg2[c][:], fjT[c][:, od],
                                         start=False, stop=(c == CT - 1),
                                         tile_position=(0, 64))
                for s in range(SJ):
                    ssl = slice(s * NCH, (s + 1) * NCH)
                    nc.scalar.activation(phi_j2[:, ssl], psj[s][:], Exp,
                                         bias=bias[:], scale=1.0)

                # ---- main matmul, 2x row-tiled; evac DVE/ACT; 2MB row DMAs ----
                nv = 0
                for m in range(MI // 128):
                    msl = slice(m * 128, (m + 1) * 128)
                    row = stage.tile([128, MJ], F32, name="row", tag="row")
                    for s in range(SJ):
                        ssl = slice(s * NCH, (s + 1) * NCH)
                        ev = slice((2 * s) * NCH, (2 * s + 1) * NCH)
                        od = slice((2 * s + 1) * NCH, (2 * s + 2) * NCH)
                        # reuse the (now free) dist_j psum banks, 4-deep rotation
                        pa = psum.tile([128, NCH], F32, name="mpsa",
                                       tag=f"dpsj{2 * (s % 2)}")
                        pb = psum.tile([128, NCH], F32, name="mpsb",
                                       tag=f"dpsj{2 * (s % 2) + 1}")
                        nc.tensor.matmul(pa[:], phi_i2[0:64, msl], phi_j2[0:64, ssl],
                                         start=True, stop=True, tile_position=(0, 0))
                        nc.tensor.matmul(pb[:], phi_i2[64:128, msl], phi_j2[64:128, ssl],
                                         start=True, stop=True, tile_position=(64, 0))
                        for dst, src in ((ev, pa), (od, pb)):
                            if nv % 8 < 5:
                                nc.vector.tensor_copy(row[:, dst], src[:])
                            else:
                                nc.scalar.copy(row[:, dst], src[:])
                            nv += 1
                    if m == 0:
                        q = MJ // 4
                        for t in range(4):
                            qsl = slice(t * q, (t + 1) * q)
                            nc.sync.dma_start(out_ext[msl, qsl], row[:, qsl])
                    elif m < split_rows + 1:
                        h = MJ // 2
                        nc.sync.dma_start(out_ext[msl, 0:h], row[:, 0:h])
                        nc.sync.dma_start(out_ext[msl, h:MJ], row[:, h:MJ])
                    else:
                        nc.sync.dma_start(out_ext[msl, :], row[:])

            if iters == 1:
                body()
            else:
                engines = (mybir.EngineType.PE, mybir.EngineType.Activation,
                           mybir.EngineType.DVE, mybir.EngineType.SP)
                with tc.For_i(0, iters, 1, hint_engines=engines):
                    body()

    nc.compile()
    return nc


def shard_inputs(f_i, f_j, means, scales, weights):
    """Host-side layout prep: transpose, bf16-round, slice per core."""
    f_i = np.asarray(f_i, dtype=np.float32)
    f_j = np.asarray(f_j, dtype=np.float32)
    fiT = np.ascontiguousarray(f_i.T).astype(FP8_NP)    # [C, N]
    fjT = np.ascontiguousarray(f_j.T).astype(FP8_NP)
    meansT = np.asarray(means, dtype=np.float32).T      # [C, K]
    scalesT = np.asarray(scales, dtype=np.float32).T
    # retile [C, K] -> [128, CT*K] with the 4 c-chunks along the free dim
    meansT2 = np.ascontiguousarray(
        meansT.reshape(CT, 128, K).transpose(1, 0, 2).reshape(128, CT * K))
    scalesT2 = np.ascontiguousarray(
        scalesT.reshape(CT, 128, K).transpose(1, 0, 2).reshape(128, CT * K))
    wcol = np.asarray(weights, dtype=np.float32).reshape(K, 1)
    w2 = np.concatenate([wcol, wcol], axis=0)                      # [128, 1]
    small = np.ascontiguousarray(np.concatenate([meansT2, scalesT2, w2], axis=1))
    in_maps = []
    for p in range(8):
        ir, jc = p // Q, p % Q
        in_maps.append({
            "fiT": np.ascontiguousarray(fiT[:, ir * MI:(ir + 1) * MI]),
            "fjT": np.ascontiguousarray(fjT[:, jc * MJ:(jc + 1) * MJ]),
            "small": small,
        })
    return in_maps


def assemble_output(results):
    out = np.empty((N, N), dtype=np.float32)
    for p in range(8):
        ir, jc = p // Q, p % Q
        out[ir * MI:(ir + 1) * MI, jc * MJ:(jc + 1) * MJ] = results[p]["out"]
    return out


_NC_CACHE = {}


def get_nc(iters: int = 1):
    if iters not in _NC_CACHE:
        _NC_CACHE[iters] = build_nc(iters)
    return _NC_CACHE[iters]


def kernel(f_i, f_j, means, scales, weights):
    nc = get_nc(1)
    in_maps = shard_inputs(f_i, f_j, means, scales, weights)
    try:
        res = run_bass_kernel_spmd(nc, in_maps, core_ids=list(range(8)))
    except Exception:
        # transient device-unrecoverable states have been observed right
        # after heavy benchmarking sessions; one retry after a pause
        import time as _time
        _time.sleep(20)
        res = run_bass_kernel_spmd(nc, in_maps, core_ids=list(range(8)))
    return assemble_output(res.results)

